# revision 24
# baseline (speedup 1.0000x reference)
"""
Trainium2 Bass kernel for Llama GQA decode attention (B=8, Q=4, H=4096,
32 Q-heads / 8 KV-heads, HD=128, S=4096 cached tokens, fp32).

Sharding: tensor-parallel over heads across 8 cores. Core c owns KV head c
and its 4 query heads: Wq/Wk/Wv column slices, Wo row slice, K/V cache
kv-head slice. Each core computes a partial [32, 4096] output (its heads'
contribution through Wo); the full output is the sum over cores (done on
host -- no collectives needed).

All hot matmuls are arranged stream-heavy (small stationary operand, large
moving operand) so the tensor engine is streaming-bound, not
LDWEIGHTS-bound:
    q/k/v proj:  lhsT=hsT tile [128,32] (ldw 32)  rhs=W tile   (stream <=512)
    scores:      lhsT=qT_b   [128,16]  (ldw 16)   rhs=KT chunk (stream 512)
    p @ V:       lhsT=pT tile [128,16] (ldw 16)   rhs=V tile   (stream 128)
    out proj:    lhsT=oT g-slice [128,32] (ldw 32) rhs=Wo tile (stream 512)
The K-cache shard is host-pre-transposed to [d, s] so score chunks stream
straight from DRAM. Softmax runs along the free dim; exp's accum_out
computes row sums for free. p is transposed on the (otherwise idle) DVE via
32x32 block-transposes + 4 multi-tile scatter copies per batch. RoPE's
rotate-half is a PE matmul against a constant +-1 rotation matrix.

New tokens never touch the DRAM cache: their K/V stay in SBUF and are
attended to separately with the causal triangle mask; positions >=
cache_len in the DRAM cache are never read (tiles fully beyond cache_len
are skipped, invalid tail columns of the boundary chunk get a -1e30
memset before exp).
"""

import os
import sys

sys.path.insert(0, "/opt/trn_rl_repo")

import numpy as np

import concourse.bass as bass  # noqa: F401
import concourse.tile as tile
from concourse import bacc, bass_utils, mybir

# Problem constants (hardcoded per contract)
B, Q, H = 8, 4, 4096
NH, NKV, HD = 32, 8, 128
G = NH // NKV            # 4 query heads per kv head
S = 4096                 # cache token capacity actually used
TOK = B * Q              # 32 total new tokens
GQ = G * Q               # 16 (head, query) pairs per batch
DC = G * HD              # 512 = per-core slice of the o/q head dim
N_CORES = 8
SCALE = 1.0 / (HD ** 0.5)
NEG = -1.0e30
CHUNK = 512              # score-matmul streaming chunk (s positions)

FP32 = mybir.dt.float32
FP16 = mybir.dt.float16
FP8 = mybir.dt.float8e4
Exp = mybir.ActivationFunctionType.Exp


def _build_program(nts: tuple, rems: tuple):
    """Build + compile the Bass program, specialized on per-batch cached-tile
    counts `nts` (128-tiles) and boundary-tile valid-row counts `rems`."""
    nc = bacc.Bacc("TRN2", target_bir_lowering=False, debug=False,
                   num_devices=N_CORES)

    hsT_d = nc.dram_tensor("hsT", [128, H // 128, TOK], FP16, kind="ExternalInput").ap()
    cosT_d = nc.dram_tensor("cosT", [HD, TOK], FP32, kind="ExternalInput").ap()
    sinT_d = nc.dram_tensor("sinT", [HD, TOK], FP32, kind="ExternalInput").ap()
    wq_d = nc.dram_tensor("wq", [H, DC], FP16, kind="ExternalInput").ap()
    wk_d = nc.dram_tensor("wk", [H, HD], FP16, kind="ExternalInput").ap()
    wv_d = nc.dram_tensor("wv", [H, HD], FP16, kind="ExternalInput").ap()
    wo_d = nc.dram_tensor("wo", [DC, H], FP16, kind="ExternalInput").ap()
    kT_d = nc.dram_tensor("kT", [B, HD, S], FP8, kind="ExternalInput").ap()
    v_d = nc.dram_tensor("v", [B, 128, S // 128, HD + 1], FP16, kind="ExternalInput").ap()
    rt_d = nc.dram_tensor("rt", [HD, HD], FP32, kind="ExternalInput").ap()
    mnew_d = nc.dram_tensor("mnew", [GQ, Q], FP32, kind="ExternalInput").ap()
    mbnd_d = nc.dram_tensor("mbnd", [B, 128, GQ], FP32,
                            kind="ExternalInput").ap()
    id16_d = nc.dram_tensor("id16", [GQ, GQ], FP16, kind="ExternalInput").ap()
    out_d = nc.dram_tensor("out", [TOK, H], FP32, kind="ExternalOutput").ap()

    KT = 32  # number of 128-row contraction tiles over H

    with tile.TileContext(nc) as tc:
        consts = tc.alloc_tile_pool(name="consts", bufs=1)
        wpool = tc.alloc_tile_pool(name="wtiles", bufs=4)
        kvpool = tc.alloc_tile_pool(name="kv", bufs=6)
        ppool = tc.alloc_tile_pool(name="pbuf", bufs=3)
        work = tc.alloc_tile_pool(name="work", bufs=1)
        ps_a = tc.alloc_tile_pool(name="ps_a", bufs=1, space="PSUM")
        ps_sc = tc.alloc_tile_pool(name="ps_sc", bufs=3, space="PSUM")
        ps_o = tc.alloc_tile_pool(name="ps_o", bufs=2, space="PSUM")

        # ---- constants / small inputs ----
        hsT_sb = consts.tile([128, KT, TOK], FP16)
        nc.sync.dma_start(out=hsT_sb, in_=hsT_d)
        cosT_sb = consts.tile([HD, TOK], FP32)
        nc.sync.dma_start(out=cosT_sb, in_=cosT_d)
        sinT_sb = consts.tile([HD, TOK], FP32)
        nc.sync.dma_start(out=sinT_sb, in_=sinT_d)
        rt_sb = consts.tile([HD, HD], FP32)
        nc.sync.dma_start(out=rt_sb, in_=rt_d)
        mnew_sb = consts.tile([GQ, Q], FP32)
        nc.sync.dma_start(out=mnew_sb, in_=mnew_d)
        id16_sb = consts.tile([GQ, GQ], FP16)
        nc.sync.dma_start(out=id16_sb, in_=id16_d)

        # ---- phase 1: QKV projections (natural orientation, stream-heavy) --
        # weights arrive in few ~1MB DMAs for full DMA bandwidth
        qn_ps = ps_a.tile([TOK, DC], FP32, tag="qn")   # [tok, (g, d)]
        kn_ps = ps_a.tile([TOK, HD], FP32, tag="kn")   # [tok, d]
        vn_ps = ps_a.tile([TOK, HD], FP32, tag="vn")   # [tok, d]
        KB = 8  # k-tiles per Wq DMA batch
        wq_ts = []
        for kb in range(KT // KB):
            wq_t = wpool.tile([128, KB, DC], FP16, tag="wq",
                              name=f"wq_t{kb}")
            nc.sync.dma_start(
                out=wq_t,
                in_=wq_d[kb * KB * 128:(kb + 1) * KB * 128, :]
                .rearrange("(t p) n -> p t n", p=128))
            wq_ts.append(wq_t)
            if kb == 0:
                wk_t = work.tile([128, KT, HD], FP16)
                nc.sync.dma_start(
                    out=wk_t, in_=wk_d.rearrange("(t p) n -> p t n", p=128))
                wv_t = work.tile([128, KT, HD], FP16)
                nc.sync.dma_start(
                    out=wv_t, in_=wv_d.rearrange("(t p) n -> p t n", p=128))
        for kb in range(KT // KB):
            wq_t = wq_ts[kb]
            for k8 in range(KB):
                k = kb * KB + k8
                hs_k = hsT_sb[:, k, :]
                st, sp = (k == 0), (k == KT - 1)
                nc.tensor.matmul(qn_ps, hs_k, wq_t[:, k8, :],
                                 start=st, stop=sp)
                nc.tensor.matmul(kn_ps, hs_k, wk_t[:, k, :],
                                 start=st, stop=sp)
                nc.tensor.matmul(vn_ps, hs_k, wv_t[:, k, :],
                                 start=st, stop=sp)

        # ---- phase 2: transpose q/k to [d, tok-ish] layouts + RoPE ----
        qn_sb = work.tile([TOK, DC], FP32)
        nc.vector.tensor_copy(qn_sb, qn_ps)
        kn_sb = work.tile([TOK, HD], FP32)
        nc.vector.tensor_copy(kn_sb, kn_ps)
        v_sb = work.tile([TOK, HD + 1], FP16)
        nc.vector.tensor_copy(v_sb[:, 0:HD], vn_ps)
        nc.vector.memset(v_sb[:, HD:HD + 1], 1.0)

        # DVE 32x32 block transposes + scatter copies.
        # qT0 cols ordered (b, g, qi): batch slices are contiguous.
        qbt_sb = work.tile([TOK, DC], FP32)    # blockwise-transposed q
        for g in range(G):
            nc.vector.transpose(qbt_sb[:, g * HD:(g + 1) * HD],
                                qn_sb[:, g * HD:(g + 1) * HD])
        kbt_sb = work.tile([TOK, HD], FP32)
        nc.vector.transpose(kbt_sb, kn_sb)

        qT0_sb = work.tile([128, B * GQ], FP32)   # [d, (b, g, qi)]
        qT0_v = qT0_sb.rearrange("p (b g q) -> p b g q", b=B, g=G)
        qbt_v = qbt_sb.rearrange("n (g c i) -> n g c i", g=G, c=4)
        for g in range(G):
            for c in range(4):
                # qT0[c*32+i, (b, g, qi)] = qbt[i (part), (g, c, tok) free]
                nc.vector.tensor_copy(
                    qT0_v[c * 32:(c + 1) * 32, :, g, :],
                    qbt_v[:, g, c, :].rearrange("n (b q) -> n b q", b=B))
        kT0_sb = work.tile([128, TOK], FP32)      # [d, tok]
        kbt_v = kbt_sb.rearrange("n (c i) -> n c i", c=4)
        for c in range(4):
            nc.vector.tensor_copy(kT0_sb[c * 32:(c + 1) * 32, :],
                                  kbt_v[:, c, :])

        # RoPE: rotate-half via PE permutation matmul, then cos/sin combine
        qrot_ps = ps_a.tile([128, B * GQ], FP32, tag="qn")
        nc.tensor.matmul(qrot_ps, rt_sb, qT0_sb, start=True, stop=True)
        krot_ps = ps_a.tile([128, TOK], FP32, tag="kn")
        nc.tensor.matmul(krot_ps, rt_sb, kT0_sb, start=True, stop=True)

        # cos/sin for qT0 layout: value depends on (d, b, qi); bcast over g
        cos_q = bass.AP(tensor=cosT_sb.tensor, offset=cosT_sb.offset,
                        ap=[cosT_sb.ap[0], [Q, B], [0, G], [1, Q]])
        sin_q = bass.AP(tensor=sinT_sb.tensor, offset=sinT_sb.offset,
                        ap=[sinT_sb.ap[0], [Q, B], [0, G], [1, Q]])
        qf_sb = work.tile([128, B, GQ], FP8)       # rope'd qT
        qf_gq = qf_sb.rearrange("p b (g q) -> p b g q", g=G)
        tmpq_sb = work.tile([128, B, G, Q], FP32)
        q3 = qT0_sb.rearrange("p (b g q) -> p b g q", b=B, g=G)
        qr3 = qrot_ps.rearrange("p (b g q) -> p b g q", b=B, g=G)
        nc.vector.tensor_mul(tmpq_sb, q3, cos_q)
        nc.vector.tensor_mul(qf_gq, qr3, sin_q)
        nc.vector.tensor_add(qf_gq, qf_gq, tmpq_sb)

        kf_sb = work.tile([128, TOK], FP8)        # rope'd kT
        tmpk_sb = work.tile([128, TOK], FP32)
        nc.vector.tensor_mul(tmpk_sb, kT0_sb, cosT_sb)
        nc.vector.tensor_mul(kf_sb, krot_ps, sinT_sb)
        nc.vector.tensor_add(kf_sb, kf_sb, tmpk_sb)

        qf_flat = qf_sb.rearrange("p b m -> p (b m)")

        # ---- phase 3: attention per batch ----
        o_all_sb = work.tile([GQ, B, HD], FP16)   # scaled o, [gq, b, d]
        for b in range(B):
            nt = nts[b]
            ln = (nt - 1) * 128 + rems[b] if nt > 0 else 0  # cache length
            nch = (nt * 128 + CHUNK - 1) // CHUNK           # score chunks
            qf_b = qf_flat[:, b * GQ:(b + 1) * GQ]          # [128, 16]
            pT_sb = ppool.tile([128, max(nt, 1) * GQ], FP16, tag="pT")
            pT_v = pT_sb.rearrange("p (t m) -> p t m", m=GQ)

            if nt > 0:
                kT_b = kvpool.tile([128, nt * 128], FP8, tag="kT")
                nc.sync.dma_start(out=kT_b, in_=kT_d[b, :, :nt * 128])
                v_b = kvpool.tile([128, nt, HD + 1], FP16, tag="v")
                nc.sync.dma_start(out=v_b, in_=v_d[b, :, :nt, :])

                # scoresT[s, gq] per 128-tile straight from the PE: the fp16
                # 128-col LDWEIGHTS runs under FWL (~2 elem/cycle)
                scT_ps = ps_sc.tile([128, max(nt, 1) * GQ], FP32, tag="sc")
                for t in range(nt):
                    nc.tensor.matmul(scT_ps[:, t * GQ:(t + 1) * GQ],
                                     kT_b[:, t * 128:(t + 1) * 128], qf_b,
                                     start=(t == 0), stop=(t == nt - 1))
                if rems[b] < 128:  # mask invalid tail rows of last tile
                    mb_sb = ppool.tile([128, GQ], FP32, tag="mb")
                    nc.sync.dma_start(out=mb_sb, in_=mbnd_d[b])
                    nc.vector.tensor_add(
                        scT_ps[:, (nt - 1) * GQ:nt * GQ],
                        scT_ps[:, (nt - 1) * GQ:nt * GQ], mb_sb)
                nc.scalar.activation(pT_sb, scT_ps[:, :nt * GQ], Exp)

            # new-token scores [gq, jj], causal triangle mask
            sn_ps = ps_o.tile([GQ, Q], FP32, tag="o")
            nc.tensor.matmul(sn_ps, qf_b, kf_sb[:, b * Q:(b + 1) * Q],
                             start=True, stop=True)
            nc.vector.tensor_add(sn_ps, sn_ps, mnew_sb)
            pn_sb = ppool.tile([TOK, TOK], FP16, tag="pn")
            nc.gpsimd.memset(pn_sb, 0.0)
            nc.scalar.activation(pn_sb[:GQ, :Q], sn_ps, Exp)
            pnt_sb = ppool.tile([TOK, TOK], FP16, tag="pnt")
            nc.vector.transpose(pnt_sb, pn_sb)
            vb_sb = ppool.tile([Q, HD + 1], FP16, tag="vb")
            nc.sync.dma_start(out=vb_sb, in_=v_sb[b * Q:(b + 1) * Q, :])

            # o[gq, 0:128] accumulation; col 128 accumulates the softmax
            # denominator via V's ones column
            o_ps = ps_o.tile([GQ, HD + 1], FP32, tag="o")
            if nt > 0:
                for t in range(nt):
                    nc.tensor.matmul(o_ps, pT_sb[:, t * GQ:(t + 1) * GQ],
                                     v_b[:, t, :],
                                     start=(t == 0), stop=False)
            nc.tensor.matmul(o_ps, pnt_sb[:Q, :GQ], vb_sb,
                             start=(nt == 0), stop=True)
            rec_sb = ppool.tile([GQ, 1], FP32, tag="rec")
            nc.vector.reciprocal(rec_sb, o_ps[:, HD:HD + 1])
            nc.vector.tensor_scalar_mul(o_all_sb[:, b, :], o_ps[:, 0:HD],
                                        rec_sb)

        # ---- transpose o -> oT [d, (g, b, qi)] via PE + one reorder copy --
        oT_ps = ps_a.tile([128, B, GQ], FP32, tag="vn")
        for b in range(B):
            nc.tensor.matmul(oT_ps[:, b, :], o_all_sb[:, b, :], id16_sb,
                             start=True, stop=True)
        oT_sb = work.tile([128, G, B, Q], FP16)
        nc.vector.tensor_copy(
            oT_sb,
            oT_ps.rearrange("p b (g q) -> p g b q", g=G))

        # ---- phase 4: output projection (Wo resident in 4 ~1MB tiles) ----
        out_sb = work.tile([TOK, H], FP32)
        oT_flat = oT_sb.rearrange("p g b q -> p (g b q)")
        wo_ts = []
        for g in range(G):
            wo_g = wpool.tile([128, H], FP16, tag="wq", name=f"wo_{g}")
            nc.sync.dma_start(out=wo_g, in_=wo_d[g * HD:(g + 1) * HD, :])
            wo_ts.append(wo_g)
        NCH = 8  # 512-wide chunks of H
        for n in range(NCH):
            fo_ps = ps_sc.tile([TOK, 512], FP32, tag="sc")
            for g in range(G):
                nc.tensor.matmul(fo_ps, oT_flat[:, g * TOK:(g + 1) * TOK],
                                 wo_ts[g][:, n * 512:(n + 1) * 512],
                                 start=(g == 0), stop=(g == G - 1))
            nc.vector.tensor_copy(out_sb[:, n * 512:(n + 1) * 512], fo_ps)
            nc.sync.dma_start(out=out_d[:, n * 512:(n + 1) * 512],
                              in_=out_sb[:, n * 512:(n + 1) * 512])

        ps_o.release()
        ps_sc.release()
        ps_a.release()
        work.release()
        ppool.release()
        kvpool.release()
        wpool.release()
        consts.release()

    nc.compile()
    return nc


_PROGRAM_CACHE: dict = {}


def _get_program(nts, rems):
    key = (tuple(nts), tuple(rems))
    if key not in _PROGRAM_CACHE:
        _PROGRAM_CACHE[key] = _build_program(tuple(nts), tuple(rems))
    return _PROGRAM_CACHE[key]


def _prep_inputs(hidden_states, cos, sin, Wq, Wk, Wv, Wo, K_cache, V_cache,
                 cache_lens):
    """Host-side shard prep. Returns (in_maps, nts, rems)."""
    f32 = np.float32
    f16 = np.float16
    # hsT tiled: hs3[p, t, n] = hs[n, t*128 + p]
    hs = np.ascontiguousarray(
        hidden_states.reshape(TOK, H).T.reshape(H // 128, 128, TOK)
        .transpose(1, 0, 2), dtype=f16)
    cosT = np.ascontiguousarray(cos.reshape(TOK, HD).T, dtype=f32)
    sinT = np.ascontiguousarray(sin.reshape(TOK, HD).T, dtype=f32)

    lens = np.asarray(cache_lens, dtype=np.int64)
    nts, rems = [], []
    for b in range(B):
        ln = int(min(max(lens[b], 0), S))
        nt = (ln + 127) // 128
        rem = ln - (nt - 1) * 128 if nt > 0 else 128
        nts.append(nt)
        rems.append(rem)

    # rotate-half matrix R (with sign), transposed for lhsT use:
    # rot[d'] = -q[d'+64] for d'<64 ; +q[d'-64] for d'>=64
    R = np.zeros((HD, HD), dtype=f32)
    hh = HD // 2
    for dp in range(hh):
        R[dp, dp + hh] = -1.0
        R[dp + hh, dp] = 1.0
    rt = np.ascontiguousarray(R.T)

    # new-token causal triangle: query qi sees new position jj iff jj <= qi
    mnew = np.zeros((GQ, Q), dtype=f32)
    for g in range(G):
        for qi in range(Q):
            for jj in range(Q):
                if jj > qi:
                    mnew[g * Q + qi, jj] = NEG

    id16 = np.eye(GQ, dtype=f16)

    # boundary masks: rows >= rem of a batch's last cached tile are invalid
    mbnd = np.zeros((B, 128, GQ), dtype=f32)
    for b in range(B):
        if nts[b] > 0 and rems[b] < 128:
            mbnd[b, rems[b]:, :] = NEG

    in_maps = []
    for c in range(N_CORES):
        wq = (np.asarray(Wq[:, c * DC:(c + 1) * DC], dtype=f32)
              * f32(SCALE)).astype(f16)
        wk = np.ascontiguousarray(Wk[:, c * HD:(c + 1) * HD], dtype=f16)
        wv = np.ascontiguousarray(Wv[:, c * HD:(c + 1) * HD], dtype=f16)
        wo = np.ascontiguousarray(Wo[c * DC:(c + 1) * DC, :], dtype=f16)
        kT = np.ascontiguousarray(
            K_cache[:, :S, c, :].transpose(0, 2, 1)).astype(
                mybir.dt.np(FP8))
        # v tiled + ones column: v4[b, p, t, 0:128] = V[b, t*128+p, :],
        # v4[b, p, t, 128] = 1.0 (accumulates softmax denominators)
        v = np.empty((B, 128, S // 128, HD + 1), dtype=f16)
        v[..., 0:HD] = (np.asarray(V_cache[:, :S, c, :], dtype=np.float32)
                        .reshape(B, S // 128, 128, HD).transpose(0, 2, 1, 3))
        v[..., HD] = 1.0
        in_maps.append(dict(hsT=hs, cosT=cosT, sinT=sinT, wq=wq, wk=wk,
                            wv=wv, wo=wo, kT=kT, v=v, rt=rt, mnew=mnew,
                            id16=id16, mbnd=mbnd))
    return in_maps, nts, rems


def _install_axon_ntff_hook():
    """The agent image's antenv lacks axon_hooks; recreate the NTFF profile
    hook via ctypes against libaxon_pjrt.so so trace=True yields exec times."""
    try:
        from antenv.axon_hooks import get_axon_ntff_profile_hook  # noqa: F401
        return
    except ImportError:
        pass
    import contextlib
    import ctypes
    import types

    so_path = "/opt/axon/libaxon_pjrt.so"
    try:
        lib = ctypes.CDLL(so_path)
    except OSError:
        return
    if not hasattr(lib, "axon_start_nrt_profile"):
        return
    lib.axon_start_nrt_profile.argtypes = [ctypes.POINTER(ctypes.c_int64),
                                           ctypes.c_size_t]
    lib.axon_start_nrt_profile.restype = ctypes.c_int64
    lib.axon_stop_nrt_profile.argtypes = [ctypes.c_char_p]
    lib.axon_stop_nrt_profile.restype = ctypes.c_int64

    @contextlib.contextmanager
    def _hook(output_dir, device_ids):
        import jax
        jax.devices()
        if device_ids:
            ids = (ctypes.c_int64 * len(device_ids))(*device_ids)
            rc = lib.axon_start_nrt_profile(ids, len(device_ids))
        else:
            rc = lib.axon_start_nrt_profile(None, 0)
        if rc != 0:
            raise RuntimeError(f"axon_start_nrt_profile rc={rc}")
        try:
            yield
        finally:
            n = lib.axon_stop_nrt_profile(str(output_dir).encode())
            if n <= 0:
                print(f"profile: rc={n} writing to {output_dir}",
                      file=sys.stderr)

    import antenv
    mod = types.ModuleType("antenv.axon_hooks")
    mod.get_axon_ntff_profile_hook = lambda: _hook
    mod.set_axon_ntff_profile_hook = lambda h: None
    sys.modules["antenv.axon_hooks"] = mod
    antenv.axon_hooks = mod


_LAST_RESULTS = {}


def kernel(hidden_states, cos, sin, Wq, Wk, Wv, Wo, K_cache, V_cache,
           cache_lens):
    in_maps, nts, rems = _prep_inputs(hidden_states, cos, sin, Wq, Wk, Wv,
                                      Wo, K_cache, V_cache, cache_lens)
    nc = _get_program(nts, rems)

    trace = bool(int(os.environ.get("BASS_KERNEL_TRACE", "0")))
    if trace:
        _install_axon_ntff_hook()
    res = bass_utils.run_bass_kernel_spmd(
        nc, in_maps, core_ids=list(range(N_CORES)), trace=trace)
    _LAST_RESULTS["res"] = res

    total = np.zeros((TOK, H), dtype=np.float64)
    for c in range(N_CORES):
        total += res.results[c]["out"].astype(np.float64)
    return total.astype(np.float32).reshape(B, Q, H)
